# revision 1
# baseline (speedup 1.0000x reference)
# MLA (Multi-head Latent Attention) Trainium2 kernel, 8-core SPMD.
#
# Sharding: data-parallel over batch (B=2) x tensor-parallel over heads
# (16 heads -> 4 groups of 4). Core c handles batch c//4, heads 4*(c%4)..+4.
# Each core computes the full down-projections for its batch (duplicated
# across the 4 cores of the batch group), up-projects only its own heads,
# runs causal attention, and produces a row-parallel partial of the output
# projection. The host sums the 4 partials per batch and adds b_o.
#
# All matmuls run in float32r (TF32-like, ~1e-4 rel err, full PE speed).
# Attention computes scores TRANSPOSED ([k, q]) so exp(scores) is directly
# the P^T operand PV needs; softmax denominators come from a ones-vector
# matmul on the PE and normalization happens on eviction.  No max
# subtraction: |scores|*scale is bounded (~5) for any plausible input, so
# exp cannot overflow.
import numpy as np
from contextlib import ExitStack

B, S, HID = 2, 2048, 2048
NH, HD, RD = 16, 128, 64
KVC, QC = 512, 1536
NCORES = 8
HPC = 4                 # heads per core
SCALE = 1.0 / float(np.sqrt(HD + RD))

_CACHE = {}


def _build_nc(repeat=1, upto=5):
    import concourse.bacc as bacc
    import concourse.mybir as mybir
    import concourse.tile as tile

    F32R = mybir.dt.float32r
    F32 = mybir.dt.float32
    AF = mybir.ActivationFunctionType

    nc = bacc.Bacc("TRN2", target_bir_lowering=False, debug=False)

    xT = nc.dram_tensor("xT", [HID, S], F32R, kind="ExternalInput")
    w_kvd = nc.dram_tensor("w_kvd", [HID, KVC], F32R, kind="ExternalInput")
    w_qd = nc.dram_tensor("w_qd", [HID, QC], F32R, kind="ExternalInput")
    w_ku = nc.dram_tensor("w_ku", [KVC, HPC * HD], F32R, kind="ExternalInput")
    w_vu = nc.dram_tensor("w_vu", [KVC, HPC * HD], F32R, kind="ExternalInput")
    w_kr = nc.dram_tensor("w_kr", [KVC, HPC * RD], F32R, kind="ExternalInput")
    w_qu = nc.dram_tensor("w_qu", [QC, HPC * HD], F32R, kind="ExternalInput")
    w_qr = nc.dram_tensor("w_qr", [QC, HPC * RD], F32R, kind="ExternalInput")
    w_o = nc.dram_tensor("w_o", [HPC * HD, HID], F32R, kind="ExternalInput")
    b_kvd = nc.dram_tensor("b_kvd", [128, 4], F32, kind="ExternalInput")
    b_qd = nc.dram_tensor("b_qd", [128, 12], F32, kind="ExternalInput")
    b_ku = nc.dram_tensor("b_ku", [128, 4], F32, kind="ExternalInput")
    b_kr = nc.dram_tensor("b_kr", [128, 2], F32, kind="ExternalInput")
    b_qu = nc.dram_tensor("b_qu", [128, 4], F32, kind="ExternalInput")
    b_qr = nc.dram_tensor("b_qr", [128, 2], F32, kind="ExternalInput")
    b_vu = nc.dram_tensor("b_vu", [1, HPC * HD], F32R, kind="ExternalInput")
    cospair = nc.dram_tensor("cospair", [128, S], F32R, kind="ExternalInput")
    sinpair = nc.dram_tensor("sinpair", [128, S], F32R, kind="ExternalInput")
    causal = nc.dram_tensor("causal", [128, 128], F32, kind="ExternalInput")
    out_p = nc.dram_tensor("out_p", [S, HID], F32, kind="ExternalOutput")

    NB = S // 128        # 16 seq blocks
    with tile.TileContext(nc) as tc:
        with ExitStack() as sa:   # whole-kernel scope
            dram = sa.enter_context(tc.tile_pool(name="dram", bufs=1, space="DRAM"))
            qcT_d = dram.tile([QC, S], F32R)
            qT_d = dram.tile([HPC * 128, S], F32R)

            consts = sa.enter_context(tc.tile_pool(name="consts", bufs=1))
            ones_f = consts.tile([1, 128], F32, tag="onesf")
            nc.vector.memset(ones_f[:], 1.0)
            ones = consts.tile([1, 128], F32R, tag="ones")
            nc.vector.tensor_copy(ones[:], ones_f[:])
            onesc_f = consts.tile([128, 1], F32, tag="onescf")
            nc.vector.memset(onesc_f[:], 1.0)
            onesc = consts.tile([128, 1], F32R, tag="onesc")
            nc.vector.tensor_copy(onesc[:], onesc_f[:])
            causal_t = consts.tile([128, 128], F32, tag="causal")
            nc.sync.dma_start(causal_t[:], causal.ap())
            bias_tiles = {}
            for nm, t, w in [("b_kvd", b_kvd, 4), ("b_qd", b_qd, 12),
                             ("b_ku", b_ku, 4), ("b_kr", b_kr, 2),
                             ("b_qu", b_qu, 4), ("b_qr", b_qr, 2)]:
                bt = consts.tile([128, w], F32, tag=nm, name=nm + "_t")
                nc.sync.dma_start(bt[:], t.ap())
                bias_tiles[nm] = bt
            bvu_t = consts.tile([1, HPC * HD], F32R, tag="bvu")
            nc.sync.dma_start(bvu_t[:], b_vu.ap())

            def rope_pair(raw, out, cos_t, sin_t, tmp_pool):
                # raw/out: [128, S] pair tile (rows: [h_even 64 | h_odd 64],
                # within head: [t1 32 | t2 32]).  out = raw*cos + shuf(raw)*sin
                shuf = tmp_pool.tile([128, S], F32R, tag="shuf", name="shuf")
                for a in range(4):
                    src = (a ^ 1) * 32
                    nc.sync.dma_start(shuf[a * 32:(a + 1) * 32, :],
                                      raw[src:src + 32, :])
                t1 = tmp_pool.tile([128, S], F32R, tag="ropetmp", name="ropetmp")
                nc.vector.tensor_mul(t1[:], raw[:], cos_t[:])
                nc.vector.tensor_mul(shuf[:], shuf[:], sin_t[:])
                nc.vector.tensor_add(out[:], t1[:], shuf[:])

            for _rep in range(repeat):
              with ExitStack() as srep:
                with ExitStack() as sab:  # kv_cT lives through A + B1
                    kvcT_pool = sab.enter_context(tc.tile_pool(name="kvcT", bufs=1))
                    kvcT = [kvcT_pool.tile([128, S], F32R, tag=f"kvcT{i}", name=f"kvcT{i}")
                            for i in range(KVC // 128)]

                    # ---- Phase A: down projections (kv_cT in SBUF, q_cT -> DRAM)
                    # Stationary (weight chunk) is reused across the 4 s-chunks by
                    # accumulating 4 PSUM groups in parallel.
                    with ExitStack() as s:
                        xp = s.enter_context(tc.tile_pool(name="xp", bufs=16))
                        wp = s.enter_context(tc.tile_pool(name="wA", bufs=2))
                        evp = s.enter_context(tc.tile_pool(name="evA", bufs=3))
                        ps = s.enter_context(tc.tile_pool(name="psA", bufs=2, space="PSUM"))
                        xt = []
                        for i in range(16):
                            t = xp.tile([128, S], F32R, tag="x", name="xt")
                            nc.sync.dma_start(t[:], xT.ap()[i * 128:(i + 1) * 128, :])
                            xt.append(t)
                        wkvd_r = w_kvd.ap().rearrange("(hc hp) o -> hp hc o", hp=128)
                        wqd_r = w_qd.ap().rearrange("(hc hp) o -> hp hc o", hp=128)
                        for ot in range(16):
                            wt = wp.tile([128, 16, 128], F32R, tag="w", name="wA")
                            if ot < 4:
                                nc.sync.dma_start(
                                    wt[:], wkvd_r[:, :, ot * 128:(ot + 1) * 128])
                            else:
                                o2 = ot - 4
                                nc.sync.dma_start(
                                    wt[:], wqd_r[:, :, o2 * 128:(o2 + 1) * 128])
                            pts = [ps.tile([128, 512], F32, tag=f"ps{sc}",
                                           name=f"psA{sc}") for sc in range(4)]
                            for hc in range(16):
                                for sc in range(4):
                                    nc.tensor.matmul(
                                        pts[sc][:], wt[:, hc, :],
                                        xt[hc][:, sc * 512:(sc + 1) * 512],
                                        start=(hc == 0), stop=(hc == 15))
                            for sc in range(4):
                                if ot < 4:
                                    nc.scalar.activation(
                                        kvcT[ot][:, sc * 512:(sc + 1) * 512],
                                        pts[sc][:], AF.Identity,
                                        bias=bias_tiles["b_kvd"][:, ot:ot + 1])
                                else:
                                    ev = evp.tile([128, 512], F32R, tag="ev",
                                                  name="evA")
                                    nc.scalar.activation(
                                        ev[:], pts[sc][:], AF.Identity,
                                        bias=bias_tiles["b_qd"][:, ot - 4:ot - 3])
                                    nc.sync.dma_start(
                                        qcT_d[(ot - 4) * 128:(ot - 3) * 128,
                                              sc * 512:(sc + 1) * 512], ev[:])

                    if upto >= 2:
                        # ---- Phase B1: kv-side up projections + k rope + V
                        kv_out_pool = srep.enter_context(
                            tc.tile_pool(name="kv_out", bufs=1, side="right"))
                        kT = [kv_out_pool.tile([128, S], F32R, tag=f"kT{h}", name=f"kT{h}")
                              for h in range(HPC)]
                        krT = [kv_out_pool.tile([128, S], F32R, tag=f"krT{p}", name=f"krT{p}")
                               for p in range(2)]
                        V_all = kv_out_pool.tile([128, NB * HPC * HD], F32R, tag="V",
                                                 name="V_all")
                        with ExitStack() as s:
                            wbp = s.enter_context(tc.tile_pool(name="wB1", bufs=1))
                            tmp = s.enter_context(tc.tile_pool(name="tmpB1", bufs=1))
                            ps = s.enter_context(tc.tile_pool(name="psB1", bufs=2, space="PSUM"))
                            wku_t = wbp.tile([128, 4 * 512], F32R, tag="wku")
                            nc.sync.dma_start(
                                wku_t[:].rearrange("p (cc o) -> p cc o", o=512),
                                w_ku.ap().rearrange("(cc cp) o -> cp cc o", cp=128))
                            wvu_t = wbp.tile([128, 4 * 512], F32R, tag="wvu")
                            nc.sync.dma_start(
                                wvu_t[:].rearrange("p (cc o) -> p cc o", o=512),
                                w_vu.ap().rearrange("(cc cp) o -> cp cc o", cp=128))
                            wkr_t = wbp.tile([128, 4 * 256], F32R, tag="wkr")
                            nc.sync.dma_start(
                                wkr_t[:].rearrange("p (cc o) -> p cc o", o=256),
                                w_kr.ap().rearrange("(cc cp) o -> cp cc o", cp=128))
                            cos_t = tmp.tile([128, S], F32R, tag="cos")
                            nc.sync.dma_start(cos_t[:], cospair.ap())
                            sin_t = tmp.tile([128, S], F32R, tag="sin")
                            nc.sync.dma_start(sin_t[:], sinpair.ap())

                            krraw = [tmp.tile([128, S], F32R, tag=f"krraw{p}",
                                              name=f"krraw{p}") for p in range(2)]
                            # k_c heads and k_r pairs: stationary reused over s-chunks
                            for dst, wsrc, no, ow, bias in (
                                    (kT, wku_t, HPC, 512, "b_ku"),
                                    (krraw, wkr_t, 2, 256, "b_kr")):
                                for o in range(no):
                                    pts = [ps.tile([128, 512], F32, tag=f"ps{sc}",
                                                   name=f"psB{sc}") for sc in range(4)]
                                    for cc in range(4):
                                        for sc in range(4):
                                            nc.tensor.matmul(
                                                pts[sc][:],
                                                wsrc[:, cc * ow + o * 128:
                                                     cc * ow + (o + 1) * 128],
                                                kvcT[cc][:, sc * 512:(sc + 1) * 512],
                                                start=(cc == 0), stop=(cc == 3))
                                    for sc in range(4):
                                        nc.scalar.activation(
                                            dst[o][:, sc * 512:(sc + 1) * 512],
                                            pts[sc][:], AF.Identity,
                                            bias=bias_tiles[bias][:, o:o + 1])
                            for p in range(2):
                                rope_pair(krraw[p], krT[p], cos_t, sin_t, tmp)
                            for st in range(NB):      # V (natural layout, bias via PE)
                                pt = ps.tile([128, 512], F32, tag="ps0", name="psV")
                                nc.tensor.matmul(pt[:], ones[:], bvu_t[:],
                                                 start=True, stop=False)
                                for cc in range(4):
                                    nc.tensor.matmul(
                                        pt[:], kvcT[cc][:, st * 128:(st + 1) * 128],
                                        wvu_t[:, cc * 512:(cc + 1) * 512],
                                        start=False, stop=(cc == 3))
                                nc.scalar.copy(V_all[:, st * 512:(st + 1) * 512], pt[:])

                if upto >= 3:
                    # ---- Phase B2: q-side up projections (q_cT streamed from DRAM)
                    with ExitStack() as sqr:
                        tmp2 = sqr.enter_context(tc.tile_pool(name="tmpB2", bufs=1))
                        qrraw = [tmp2.tile([128, S], F32R, tag=f"qrraw{p}",
                                           name=f"qrraw{p}") for p in range(2)]
                        with ExitStack() as s:
                            wbp = s.enter_context(tc.tile_pool(name="wB2", bufs=1))
                            qcp = s.enter_context(tc.tile_pool(name="qc", bufs=3))
                            ps = s.enter_context(tc.tile_pool(name="psB2", bufs=4, space="PSUM"))
                            wqu_t = wbp.tile([128, 12 * 512], F32R, tag="wqu")
                            nc.sync.dma_start(
                                wqu_t[:].rearrange("p (cc o) -> p cc o", o=512),
                                w_qu.ap().rearrange("(cc cp) o -> cp cc o", cp=128))
                            wqr_t = wbp.tile([128, 12 * 256], F32R, tag="wqr")
                            nc.sync.dma_start(
                                wqr_t[:].rearrange("p (cc o) -> p cc o", o=256),
                                w_qr.ap().rearrange("(cc cp) o -> cp cc o", cp=128))
                            for sc in range(4):       # 512-wide s-chunks
                                qcc = []
                                for cc in range(12):
                                    t = qcp.tile([128, 512], F32R, tag="qc", name="qcc",
                                                 bufs=24)
                                    nc.sync.dma_start(
                                        t[:], qcT_d[cc * 128:(cc + 1) * 128,
                                                    sc * 512:(sc + 1) * 512])
                                    qcc.append(t)
                                for h in range(HPC):
                                    pt = ps.tile([128, 512], F32, tag="ps", name="psB2")
                                    for cc in range(12):
                                        nc.tensor.matmul(
                                            pt[:],
                                            wqu_t[:, cc * 512 + h * 128:
                                                  cc * 512 + (h + 1) * 128],
                                            qcc[cc][:], start=(cc == 0), stop=(cc == 11))
                                    evq = qcp.tile([128, 512], F32R, tag="evq",
                                                   name="evq")
                                    nc.scalar.activation(
                                        evq[:], pt[:],
                                        AF.Identity, bias=bias_tiles["b_qu"][:, h:h + 1])
                                    nc.sync.dma_start(
                                        qT_d[h * 128:(h + 1) * 128,
                                             sc * 512:(sc + 1) * 512], evq[:])
                                for p in range(2):
                                    pt = ps.tile([128, 512], F32, tag="ps", name="psB2")
                                    for cc in range(12):
                                        nc.tensor.matmul(
                                            pt[:],
                                            wqr_t[:, cc * 256 + p * 128:
                                                  cc * 256 + (p + 1) * 128],
                                            qcc[cc][:], start=(cc == 0), stop=(cc == 11))
                                    nc.scalar.activation(
                                        qrraw[p][:, sc * 512:(sc + 1) * 512], pt[:],
                                        AF.Identity, bias=bias_tiles["b_qr"][:, p:p + 1])
                        # rope for q (separate scope so wq pools are freed first)
                        qr_out_pool = srep.enter_context(
                            tc.tile_pool(name="qr_out", bufs=1, side="right"))
                        qrT = [qr_out_pool.tile([128, S], F32R, tag=f"qrT{p}", name=f"qrT{p}")
                               for p in range(2)]
                        with ExitStack() as s:
                            tmp3 = s.enter_context(tc.tile_pool(name="tmpB2b", bufs=1))
                            cos_t = tmp3.tile([128, S], F32R, tag="cos2")
                            nc.sync.dma_start(cos_t[:], cospair.ap())
                            sin_t = tmp3.tile([128, S], F32R, tag="sin2")
                            nc.sync.dma_start(sin_t[:], sinpair.ap())
                            for p in range(2):
                                rope_pair(qrraw[p], qrT[p], cos_t, sin_t, tmp3)

                if upto >= 4:
                    # ---- Phase C: causal attention, transposed-scores formulation.
                    # scoresT[k, q] = (kT_j)^T qT + (krT_j)^T qrT; PT = exp(scale * .);
                    # ctxT[d, q] += V_j^T PT_j;  den[1, q] += ones^T PT_j;
                    # ctxT normalized by 1/den on eviction (PE broadcast of rden).
                    ctx_pool = srep.enter_context(
                        tc.tile_pool(name="ctx", bufs=1, side="right"))
                    ctxT = [ctx_pool.tile([128, S], F32R, tag=f"ctxT{h}", name=f"ctxT{h}")
                            for h in range(HPC)]
                    with ExitStack() as s:
                        qTg_p = s.enter_context(tc.tile_pool(name="qTg", bufs=3))
                        PT_p = s.enter_context(tc.tile_pool(name="PTp", bufs=4))
                        sm = s.enter_context(tc.tile_pool(name="smC", bufs=4))
                        ps_sc = s.enter_context(tc.tile_pool(name="ps_sc", bufs=3, space="PSUM"))
                        ps_cx = s.enter_context(tc.tile_pool(name="ps_cx", bufs=2, space="PSUM"))
                        ps_dn = s.enter_context(tc.tile_pool(name="ps_dn", bufs=2, space="PSUM"))
                        ps_bc = s.enter_context(tc.tile_pool(name="ps_bc", bufs=1, space="PSUM"))
                        for g in range(4):
                            for h in range(HPC):
                                pr, off = h // 2, (h % 2) * 64
                                qlo = g * 512
                                qTg = qTg_p.tile([128, 512], F32R, tag="qTg", name="qTg")
                                nc.sync.dma_start(
                                    qTg[:], qT_d[h * 128:(h + 1) * 128, qlo:qlo + 512])
                                pcx = ps_cx.tile([128, 512], F32, tag="ctx", name="pcx")
                                pden = ps_dn.tile([1, 512], F32, tag="den", name="pden")
                                njs = 4 * g + 4
                                for j in range(njs):
                                    c0 = max(0, j - 4 * g) * 128
                                    pS = ps_sc.tile([128, 512], F32, tag="sT", name="pS")
                                    nc.tensor.matmul(
                                        pS[:, c0:512],
                                        kT[h][:, j * 128:(j + 1) * 128],
                                        qTg[:, c0:512], start=True, stop=False)
                                    nc.tensor.matmul(
                                        pS[:, c0:512],
                                        krT[pr][off:off + 64, j * 128:(j + 1) * 128],
                                        qrT[pr][off:off + 64, qlo + c0:qlo + 512],
                                        start=False, stop=True)
                                    if j >= 4 * g:   # diagonal block
                                        nc.vector.tensor_add(
                                            pS[:, c0:c0 + 128], pS[:, c0:c0 + 128],
                                            causal_t[:])
                                    PTt = PT_p.tile([128, 512], F32R, tag="PT", name="PTt")
                                    nc.scalar.activation(
                                        PTt[:, c0:512], pS[:, c0:512], AF.Exp,
                                        scale=SCALE)
                                    nc.tensor.matmul(
                                        pcx[:, c0:512],
                                        V_all[:, j * 512 + h * 128:j * 512 + (h + 1) * 128],
                                        PTt[:, c0:512],
                                        start=(j == 0), stop=(j == njs - 1))
                                    nc.tensor.matmul(
                                        pden[:, c0:512], onesc[:], PTt[:, c0:512],
                                        start=(j == 0), stop=(j == njs - 1))
                                rden = sm.tile([1, 512], F32R, tag="rden", name="rden")
                                with nc.allow_low_precision(
                                        reason="softmax rdenom as f32r matmul operand"):
                                    nc.vector.reciprocal(rden[:], pden[:])
                                pbc = ps_bc.tile([128, 512], F32, tag="bc", name="pbc")
                                nc.tensor.matmul(pbc[:], ones[:], rden[:],
                                                 start=True, stop=True)
                                denb = sm.tile([128, 512], F32, tag="denb", name="denb")
                                nc.scalar.copy(denb[:], pbc[:])
                                nc.vector.tensor_mul(
                                    ctxT[h][:, qlo:qlo + 512], pcx[:], denb[:])

                if upto >= 5:
                    # ---- Phase D: output projection (row-parallel partial)
                    with ExitStack() as s:
                        wop = s.enter_context(tc.tile_pool(name="wo", bufs=1))
                        evd = s.enter_context(tc.tile_pool(name="evD", bufs=4))
                        ps = s.enter_context(tc.tile_pool(name="psD", bufs=2, space="PSUM"))
                        wo_t = [wop.tile([128, HID], F32R, tag=f"wo{h}", name=f"wo{h}")
                                for h in range(HPC)]
                        for h in range(HPC):
                            nc.sync.dma_start(
                                wo_t[h][:], w_o.ap()[h * 128:(h + 1) * 128, :])
                        for st in range(NB):
                            pts = [ps.tile([128, 512], F32, tag=f"ps{oc}",
                                           name=f"psD{oc}") for oc in range(4)]
                            for h in range(HPC):
                                for oc in range(4):
                                    nc.tensor.matmul(
                                        pts[oc][:], ctxT[h][:, st * 128:(st + 1) * 128],
                                        wo_t[h][:, oc * 512:(oc + 1) * 512],
                                        start=(h == 0), stop=(h == 3))
                            for oc in range(4):
                                ev = evd.tile([128, 512], F32, tag="evD", name="evD")
                                nc.scalar.copy(ev[:], pts[oc][:])
                                nc.sync.dma_start(
                                    out_p.ap()[st * 128:(st + 1) * 128,
                                               oc * 512:(oc + 1) * 512], ev[:])

    nc.compile()
    return nc


def _host_inputs(inputs):
    f32 = np.float32
    x = np.asarray(inputs["x"], dtype=f32)
    W_kvd, b_kvd = np.asarray(inputs["W_kvd"], f32), np.asarray(inputs["b_kvd"], f32)
    W_ku, b_ku = np.asarray(inputs["W_ku"], f32), np.asarray(inputs["b_ku"], f32)
    W_vu, b_vu = np.asarray(inputs["W_vu"], f32), np.asarray(inputs["b_vu"], f32)
    W_kr, b_kr = np.asarray(inputs["W_kr"], f32), np.asarray(inputs["b_kr"], f32)
    W_qd, b_qd = np.asarray(inputs["W_qd"], f32), np.asarray(inputs["b_qd"], f32)
    W_qu, b_qu = np.asarray(inputs["W_qu"], f32), np.asarray(inputs["b_qu"], f32)
    W_qr, b_qr = np.asarray(inputs["W_qr"], f32), np.asarray(inputs["b_qr"], f32)
    W_o = np.asarray(inputs["W_o"], f32)

    xT = [np.ascontiguousarray(x[b].T) for b in range(B)]

    inv_freq = (1.0 / (10000.0 ** (np.arange(0, RD, 2, dtype=np.float64) / RD)))
    ang = np.arange(S, dtype=np.float64)[:, None] * inv_freq[None, :]  # [S, 32]
    cosT = np.cos(ang).T.astype(f32)   # [32, S]
    sinT = np.sin(ang).T.astype(f32)
    cospair = np.ascontiguousarray(np.tile(cosT, (4, 1)))              # [128, S]
    sinpair = np.ascontiguousarray(
        np.concatenate([-sinT, sinT, -sinT, sinT], axis=0))            # [128, S]
    # transposed-scores causal mask: mask k > q within the diagonal block
    causal = np.where(np.tril(np.ones((128, 128), bool), -1),
                      f32(-1e9), f32(0.0)).astype(f32)

    in_maps = []
    for c in range(NCORES):
        b, g = c // 4, c % 4
        hc = slice(4 * g * HD, (4 * g + HPC) * HD)        # head cols (128 each)
        rc = slice(4 * g * RD, (4 * g + HPC) * RD)        # rope cols (64 each)
        m = dict(
            xT=xT[b],
            w_kvd=W_kvd, w_qd=W_qd,
            w_ku=np.ascontiguousarray(W_ku[:, hc]),
            w_vu=np.ascontiguousarray(W_vu[:, hc]),
            w_kr=np.ascontiguousarray(W_kr[:, rc]),
            w_qu=np.ascontiguousarray(W_qu[:, hc]),
            w_qr=np.ascontiguousarray(W_qr[:, rc]),
            w_o=np.ascontiguousarray(W_o[hc, :]),
            b_kvd=np.ascontiguousarray(b_kvd.reshape(4, 128).T),
            b_qd=np.ascontiguousarray(b_qd.reshape(12, 128).T),
            b_ku=np.ascontiguousarray(b_ku[hc].reshape(4, 128).T),
            b_kr=np.ascontiguousarray(b_kr[rc].reshape(2, 128).T),
            b_qu=np.ascontiguousarray(b_qu[hc].reshape(4, 128).T),
            b_qr=np.ascontiguousarray(b_qr[rc].reshape(2, 128).T),
            b_vu=np.ascontiguousarray(b_vu[hc].reshape(1, 512)),
            cospair=cospair, sinpair=sinpair, causal=causal,
        )
        in_maps.append(m)
    return in_maps, np.asarray(inputs["b_o"], f32)


def _run(inputs, trace=False):
    from concourse import bass_utils
    if "nc" not in _CACHE:
        _CACHE["nc"] = _build_nc()
    nc = _CACHE["nc"]
    in_maps, b_o = _host_inputs(inputs)
    res = bass_utils.run_bass_kernel_spmd(
        nc, in_maps, core_ids=list(range(NCORES)), trace=trace)
    out = np.zeros((B, S, HID), np.float32)
    for c in range(NCORES):
        out[c // 4] += res.results[c]["out_p"]
    out += b_o[None, None, :]
    return out, res


def kernel(**inputs) -> np.ndarray:
    out, _ = _run(inputs, trace=False)
    return out


def bench(inputs, iters=10):
    """Time NEFF execution on the 8 cores via PJRT, excluding host->device
    transfers and compile. Returns (best_ns, info)."""
    import time
    import jax
    from jax.experimental.shard_map import shard_map
    from jax.sharding import Mesh, PartitionSpec
    import concourse.mybir as mybir
    from concourse.bass2jax import (_bass_exec_p, install_neuronx_cc_hook,
                                    partition_id_tensor)

    if "nc" not in _CACHE:
        _CACHE["nc"] = _build_nc()
    nc = _CACHE["nc"]
    in_maps, _ = _host_inputs(inputs)
    install_neuronx_cc_hook()

    partition_name = nc.partition_id_tensor.name if nc.partition_id_tensor else None
    in_names, out_names, out_avals, zero_outs = [], [], [], []
    for alloc in nc.m.functions[0].allocations:
        if not isinstance(alloc, mybir.MemoryLocationSet):
            continue
        name = alloc.memorylocations[0].name
        if alloc.kind == "ExternalInput":
            if name != partition_name:
                in_names.append(name)
        elif alloc.kind == "ExternalOutput":
            out_names.append(name)
            shape = tuple(alloc.tensor_shape)
            dtype = mybir.dt.np(alloc.dtype)
            out_avals.append(jax.core.ShapedArray(shape, dtype))
            zero_outs.append(np.zeros(shape, dtype))
    n_params = len(in_names)
    all_names = list(in_names) + list(out_names)
    if partition_name is not None:
        all_names.append(partition_name)

    def _body(*args):
        operands = list(args)
        if partition_name is not None:
            operands.append(partition_id_tensor())
        outs = _bass_exec_p.bind(
            *operands,
            out_avals=tuple(out_avals),
            in_names=tuple(all_names),
            out_names=tuple(out_names),
            lowering_input_output_aliases=(),
            sim_require_finite=True,
            sim_require_nnan=True,
            nc=nc,
        )
        return tuple(outs)

    n = NCORES
    devices = jax.devices()[:n]
    mesh = Mesh(np.asarray(devices), ("core",))
    nin = n_params + len(out_names)
    fn = jax.jit(shard_map(
        _body, mesh=mesh,
        in_specs=(PartitionSpec("core"),) * nin,
        out_specs=(PartitionSpec("core"),) * len(out_names),
        check_rep=False), keep_unused=True)
    concat_in = [np.concatenate([np.asarray(in_maps[c][k]) for c in range(n)], 0)
                 for k in in_names]
    concat_zeros = [np.zeros((n * z.shape[0], *z.shape[1:]), z.dtype)
                    for z in zero_outs]
    sharding = jax.sharding.NamedSharding(mesh, PartitionSpec("core"))
    dev_in = [jax.device_put(a, sharding) for a in concat_in + concat_zeros]
    out = fn(*dev_in)  # warm-up/compile
    jax.block_until_ready(out)
    times = []
    for _ in range(iters):
        t0 = time.perf_counter()
        out = fn(*dev_in)
        jax.block_until_ready(out)
        times.append((time.perf_counter() - t0) * 1e9)
    # pipelined: K async submissions, block once; amortizes tunnel latency
    K = 10
    t0 = time.perf_counter()
    outs = [fn(*dev_in) for _ in range(K)]
    jax.block_until_ready(outs)
    tK = (time.perf_counter() - t0) * 1e9
    t0 = time.perf_counter()
    out = fn(*dev_in)
    jax.block_until_ready(out)
    t1 = (time.perf_counter() - t0) * 1e9
    piped = (tK - t1) / (K - 1)
    sustained = tK / K
    best = min(times + [sustained])
    if 0 < piped < sustained:
        best = min(best, piped)
    return best, {"serial": times, "tK": tK, "t1": t1,
                  "piped": piped, "sustained": sustained}



# revision 3
# speedup vs baseline: 4.1407x; 4.1407x over previous
# MLA (Multi-head Latent Attention) Trainium2 kernel, 8-core SPMD.
#
# Sharding: data-parallel over batch (B=2) x tensor-parallel over heads
# (16 heads -> 4 groups of 4). Core c handles batch c//4, heads 4*(c%4)..+4.
# Each core computes the full down-projections for its batch (duplicated
# across the 4 cores of the batch group), up-projects only its own heads,
# runs causal attention, and produces a row-parallel partial of the output
# projection. The host sums the 4 partials per batch and adds b_o.
#
# v2: all operands bf16 (same 1 cycle/row PE speed as f32r, half the DMA
# bytes), every intermediate SBUF-resident (no q_c/q DRAM roundtrips),
# softmax denominators accumulated on the DVE instead of per-block PE
# matmuls, weight DMA ordered ahead of x so the PE starts ~5us in, and
# bf16 output partials. Attention computes scores TRANSPOSED ([k, q]) so
# exp(scores) is directly the P^T operand PV needs; no max subtraction:
# |scores|*scale is bounded (~5) for any plausible input.
import numpy as np
from contextlib import ExitStack

B, S, HID = 2, 2048, 2048
NH, HD, RD = 16, 128, 64
KVC, QC = 512, 1536
NCORES = 8
HPC = 4                 # heads per core
SCALE = 1.0 / float(np.sqrt(HD + RD))

_CACHE = {}


def _build_nc(repeat=1, upto=5):
    import concourse.bacc as bacc
    import concourse.mybir as mybir
    import concourse.tile as tile

    BF16 = mybir.dt.bfloat16
    F32 = mybir.dt.float32
    AF = mybir.ActivationFunctionType

    nc = bacc.Bacc("TRN2", target_bir_lowering=False, debug=False)

    xT = nc.dram_tensor("xT", [HID, S], BF16, kind="ExternalInput")
    wd = nc.dram_tensor("wd", [HID, KVC + QC], BF16, kind="ExternalInput")
    wu = nc.dram_tensor("wu", [KVC, 2 * HPC * HD + HPC * RD], BF16,
                        kind="ExternalInput")
    wq = nc.dram_tensor("wq", [QC, HPC * HD + HPC * RD], BF16,
                        kind="ExternalInput")
    wo = nc.dram_tensor("wo", [HPC * HD, HID], BF16, kind="ExternalInput")
    biases = nc.dram_tensor("biases", [128, 28], F32, kind="ExternalInput")
    bvu = nc.dram_tensor("bvu", [1, HPC * HD], BF16, kind="ExternalInput")
    trig = nc.dram_tensor("trig", [2, 128, S], BF16, kind="ExternalInput")
    causal = nc.dram_tensor("causal", [128, 128], F32, kind="ExternalInput")
    out_p = nc.dram_tensor("out_p", [S, HID], BF16, kind="ExternalOutput")

    # bias column layout in `biases`
    B_KVD, B_QD, B_KU, B_KR, B_QU, B_QR = 0, 4, 16, 20, 22, 26

    NB = S // 128        # 16 seq blocks
    with tile.TileContext(nc) as tc:
        with ExitStack() as sa:   # whole-kernel scope
            consts = sa.enter_context(tc.tile_pool(name="consts", bufs=1))
            ones_f = consts.tile([1, 128], F32, tag="onesf")
            nc.vector.memset(ones_f[:], 1.0)
            ones = consts.tile([1, 128], BF16, tag="ones")
            nc.vector.tensor_copy(ones[:], ones_f[:])
            onesc_f = consts.tile([128, 1], F32, tag="onescf")
            nc.vector.memset(onesc_f[:], 1.0)
            onesc = consts.tile([128, 1], BF16, tag="onesc")
            nc.vector.tensor_copy(onesc[:], onesc_f[:])
            causal_t = consts.tile([128, 128], F32, tag="causal")
            nc.sync.dma_start(causal_t[:], causal.ap())
            bias_t = consts.tile([128, 28], F32, tag="biases")
            nc.sync.dma_start(bias_t[:], biases.ap())
            bvu_t = consts.tile([1, HPC * HD], BF16, tag="bvu")
            nc.sync.dma_start(bvu_t[:], bvu.ap())
            cos_t = consts.tile([128, S], BF16, tag="cos")
            sin_t = consts.tile([128, S], BF16, tag="sin")

            # kv/q up-projection weights, prefetched during phase A
            wub = sa.enter_context(tc.tile_pool(name="wub", bufs=1))
            wku_t = wub.tile([128, 4 * 512], BF16, tag="wku")
            wvu_t = wub.tile([128, 4 * 512], BF16, tag="wvu")
            wkr_t = wub.tile([128, 4 * 256], BF16, tag="wkr")
            wqu_t = wub.tile([128, 12 * 512], BF16, tag="wqu")
            wqr_t = wub.tile([128, 12 * 256], BF16, tag="wqr")

            def rope_pair(raw, out, tmp_pool):
                # raw/out: [128, S] pair tile (rows: [h_even 64 | h_odd 64],
                # within head: [t1 32 | t2 32]).  out = raw*cos + shuf(raw)*sin
                shuf = tmp_pool.tile([128, S], BF16, tag="shuf", name="shuf")
                for a in range(4):
                    src = (a ^ 1) * 32
                    nc.sync.dma_start(shuf[a * 32:(a + 1) * 32, :],
                                      raw[src:src + 32, :])
                t1 = tmp_pool.tile([128, S], BF16, tag="ropetmp", name="ropetmp")
                nc.vector.tensor_mul(t1[:], raw[:], cos_t[:])
                nc.vector.tensor_mul(shuf[:], shuf[:], sin_t[:])
                nc.vector.tensor_add(out[:], t1[:], shuf[:])

            for _rep in range(repeat):
              with ExitStack() as srep:
                with ExitStack() as sab:  # kv_cT + q_cT live through A..B2
                    kvq_pool = sab.enter_context(tc.tile_pool(name="kvq", bufs=1))
                    kvcT = [kvq_pool.tile([128, S], BF16, tag=f"kvcT{i}",
                                          name=f"kvcT{i}") for i in range(4)]
                    qcT = [kvq_pool.tile([128, S], BF16, tag=f"qcT{i}",
                                         name=f"qcT{i}") for i in range(12)]

                    # ---- Phase A: down projections, all SBUF-resident.
                    # Weight chunks double-buffered; first two issued before x
                    # so the PE starts as soon as x tile 0 lands.
                    with ExitStack() as s:
                        xp = s.enter_context(tc.tile_pool(name="xp", bufs=16))
                        wp = s.enter_context(tc.tile_pool(name="wA", bufs=3))
                        ps = s.enter_context(tc.tile_pool(name="psA", bufs=2, space="PSUM"))
                        wd_r = wd.ap().rearrange("(hc hp) o -> hp hc o", hp=128)
                        wts = []

                        def _issue_w(ot):
                            wt = wp.tile([128, 16, 128], BF16, tag="w", name="wA")
                            nc.sync.dma_start(wt[:], wd_r[:, :, ot * 128:(ot + 1) * 128])
                            wts.append(wt)

                        _issue_w(0)
                        _issue_w(1)
                        xt = []
                        for i in range(16):
                            t = xp.tile([128, S], BF16, tag="x", name="xt")
                            nc.sync.dma_start(t[:], xT.ap()[i * 128:(i + 1) * 128, :])
                            xt.append(t)
                        # prefetches for B1/B2 (behind x in the DMA queues)
                        nc.sync.dma_start(cos_t[:], trig.ap()[0])
                        nc.sync.dma_start(sin_t[:], trig.ap()[1])
                        nc.sync.dma_start(
                            wku_t[:].rearrange("p (cc o) -> p cc o", o=512),
                            wu.ap().rearrange("(cc cp) o -> cp cc o", cp=128)[:, :, 0:512])
                        nc.sync.dma_start(
                            wvu_t[:].rearrange("p (cc o) -> p cc o", o=512),
                            wu.ap().rearrange("(cc cp) o -> cp cc o", cp=128)[:, :, 512:1024])
                        nc.sync.dma_start(
                            wkr_t[:].rearrange("p (cc o) -> p cc o", o=256),
                            wu.ap().rearrange("(cc cp) o -> cp cc o", cp=128)[:, :, 1024:1280])
                        nc.sync.dma_start(
                            wqu_t[:].rearrange("p (cc o) -> p cc o", o=512),
                            wq.ap().rearrange("(cc cp) o -> cp cc o", cp=128)[:, :, 0:512])
                        nc.sync.dma_start(
                            wqr_t[:].rearrange("p (cc o) -> p cc o", o=256),
                            wq.ap().rearrange("(cc cp) o -> cp cc o", cp=128)[:, :, 512:768])

                        for ot in range(16):
                            if ot + 2 <= 15:
                                _issue_w(ot + 2)
                            wt = wts[ot]
                            pts = [ps.tile([128, 512], F32, tag=f"ps{sc}",
                                           name=f"psA{sc}") for sc in range(4)]
                            for hc in range(16):
                                for sc in range(4):
                                    nc.tensor.matmul(
                                        pts[sc][:], wt[:, hc, :],
                                        xt[hc][:, sc * 512:(sc + 1) * 512],
                                        start=(hc == 0), stop=(hc == 15))
                            for sc in range(4):
                                if ot < 4:
                                    nc.scalar.activation(
                                        kvcT[ot][:, sc * 512:(sc + 1) * 512],
                                        pts[sc][:], AF.Identity,
                                        bias=bias_t[:, B_KVD + ot:B_KVD + ot + 1])
                                else:
                                    nc.scalar.activation(
                                        qcT[ot - 4][:, sc * 512:(sc + 1) * 512],
                                        pts[sc][:], AF.Identity,
                                        bias=bias_t[:, B_QD + ot - 4:B_QD + ot - 3])

                    if upto >= 2:
                        # ---- Phase B1: kv-side up projections + k rope + V
                        kv_out_pool = srep.enter_context(
                            tc.tile_pool(name="kv_out", bufs=1, side="right"))
                        kT = [kv_out_pool.tile([128, S], BF16, tag=f"kT{h}",
                                               name=f"kT{h}") for h in range(HPC)]
                        krT = [kv_out_pool.tile([128, S], BF16, tag=f"krT{p}",
                                                name=f"krT{p}") for p in range(2)]
                        V_all = kv_out_pool.tile([128, NB * HPC * HD], BF16,
                                                 tag="V", name="V_all")
                        with ExitStack() as s:
                            tmp = s.enter_context(tc.tile_pool(name="tmpB1", bufs=1))
                            ps = s.enter_context(tc.tile_pool(name="psB1", bufs=2, space="PSUM"))
                            krraw = [tmp.tile([128, S], BF16, tag=f"krraw{p}",
                                              name=f"krraw{p}") for p in range(2)]
                            # k_c heads and k_r pairs: stationary reused over s-chunks
                            for dst, wsrc, no, ow, bcol in (
                                    (kT, wku_t, HPC, 512, B_KU),
                                    (krraw, wkr_t, 2, 256, B_KR)):
                                for o in range(no):
                                    pts = [ps.tile([128, 512], F32, tag=f"ps{sc}",
                                                   name=f"psB{sc}") for sc in range(4)]
                                    for cc in range(4):
                                        for sc in range(4):
                                            nc.tensor.matmul(
                                                pts[sc][:],
                                                wsrc[:, cc * ow + o * 128:
                                                     cc * ow + (o + 1) * 128],
                                                kvcT[cc][:, sc * 512:(sc + 1) * 512],
                                                start=(cc == 0), stop=(cc == 3))
                                    for sc in range(4):
                                        nc.scalar.activation(
                                            dst[o][:, sc * 512:(sc + 1) * 512],
                                            pts[sc][:], AF.Identity,
                                            bias=bias_t[:, bcol + o:bcol + o + 1])
                            for p in range(2):
                                rope_pair(krraw[p], krT[p], tmp)
                            for st in range(NB):      # V (natural layout, bias via PE)
                                pt = ps.tile([128, 512], F32, tag="ps0", name="psV")
                                nc.tensor.matmul(pt[:], ones[:], bvu_t[:],
                                                 start=True, stop=False)
                                for cc in range(4):
                                    nc.tensor.matmul(
                                        pt[:], kvcT[cc][:, st * 128:(st + 1) * 128],
                                        wvu_t[:, cc * 512:(cc + 1) * 512],
                                        start=False, stop=(cc == 3))
                                nc.scalar.copy(V_all[:, st * 512:(st + 1) * 512], pt[:])

                    if upto >= 3:
                        # ---- Phase B2: q-side up projections (qcT in SBUF)
                        qr_out_pool = srep.enter_context(
                            tc.tile_pool(name="qr_out", bufs=1, side="right"))
                        qT = [qr_out_pool.tile([128, S], BF16, tag=f"qT{h}",
                                               name=f"qT{h}") for h in range(HPC)]
                        qrT = [qr_out_pool.tile([128, S], BF16, tag=f"qrT{p}",
                                                name=f"qrT{p}") for p in range(2)]
                        with ExitStack() as s:
                            tmp2 = s.enter_context(tc.tile_pool(name="tmpB2", bufs=1))
                            ps = s.enter_context(tc.tile_pool(name="psB2", bufs=4, space="PSUM"))
                            qrraw = [tmp2.tile([128, S], BF16, tag=f"qrraw{p}",
                                               name=f"qrraw{p}") for p in range(2)]
                            for sc in range(4):       # 512-wide s-chunks
                                for h in range(HPC):
                                    pt = ps.tile([128, 512], F32, tag="ps", name="psB2")
                                    for cc in range(12):
                                        nc.tensor.matmul(
                                            pt[:],
                                            wqu_t[:, cc * 512 + h * 128:
                                                  cc * 512 + (h + 1) * 128],
                                            qcT[cc][:, sc * 512:(sc + 1) * 512],
                                            start=(cc == 0), stop=(cc == 11))
                                    nc.scalar.activation(
                                        qT[h][:, sc * 512:(sc + 1) * 512], pt[:],
                                        AF.Identity, bias=bias_t[:, B_QU + h:B_QU + h + 1])
                                for p in range(2):
                                    pt = ps.tile([128, 512], F32, tag="ps", name="psB2")
                                    for cc in range(12):
                                        nc.tensor.matmul(
                                            pt[:],
                                            wqr_t[:, cc * 256 + p * 128:
                                                  cc * 256 + (p + 1) * 128],
                                            qcT[cc][:, sc * 512:(sc + 1) * 512],
                                            start=(cc == 0), stop=(cc == 11))
                                    nc.scalar.activation(
                                        qrraw[p][:, sc * 512:(sc + 1) * 512], pt[:],
                                        AF.Identity, bias=bias_t[:, B_QR + p:B_QR + p + 1])
                            for p in range(2):
                                rope_pair(qrraw[p], qrT[p], tmp2)

                if upto >= 4:
                    # ---- Phase C: causal attention, transposed-scores formulation.
                    # scoresT[k, q] = (kT_j)^T qT + (krT_j)^T qrT; PT = exp(scale * .);
                    # ctxT[d, q] += V_j^T PT_j;  den[1, q] = ones^T (sum_j PT_j) with
                    # the sum accumulated on the DVE;  ctxT normalized by 1/den on
                    # eviction (PE broadcast of rden).
                    wop = srep.enter_context(
                        tc.tile_pool(name="wo", bufs=1, side="right"))
                    wo_t = [wop.tile([128, HID], BF16, tag=f"wo{h}", name=f"wo{h}")
                            for h in range(HPC)]
                    for h in range(HPC):
                        nc.sync.dma_start(wo_t[h][:], wo.ap()[h * 128:(h + 1) * 128, :])
                    ctx_pool = srep.enter_context(
                        tc.tile_pool(name="ctx", bufs=1, side="right"))
                    ctxT = [ctx_pool.tile([128, S], BF16, tag=f"ctxT{h}",
                                          name=f"ctxT{h}") for h in range(HPC)]
                    with ExitStack() as s:
                        PT_p = s.enter_context(tc.tile_pool(name="PTp", bufs=4))
                        sm = s.enter_context(tc.tile_pool(name="smC", bufs=4))
                        ps_sc = s.enter_context(tc.tile_pool(name="ps_sc", bufs=3, space="PSUM"))
                        ps_cx = s.enter_context(tc.tile_pool(name="ps_cx", bufs=2, space="PSUM"))
                        ps_dn = s.enter_context(tc.tile_pool(name="ps_dn", bufs=2, space="PSUM"))
                        ps_bc = s.enter_context(tc.tile_pool(name="ps_bc", bufs=2, space="PSUM"))

                        # Normalization is emitted one (g,h)-group late so the
                        # pden -> reciprocal -> pbc dependency chain hides
                        # behind the next group's score matmuls instead of
                        # bubbling the in-order PE queue.
                        pending = []

                        def flush_norm():
                            if not pending:
                                return
                            h, qlo, pcx, rden = pending.pop(0)
                            pbc = ps_bc.tile([128, 512], F32, tag="bc", name="pbc")
                            nc.tensor.matmul(pbc[:], ones[:], rden[:],
                                             start=True, stop=True)
                            denb = sm.tile([128, 512], F32, tag="denb", name="denb")
                            nc.scalar.copy(denb[:], pbc[:])
                            nc.vector.tensor_mul(
                                ctxT[h][:, qlo:qlo + 512], pcx[:], denb[:])

                        for g in range(4):
                            for h in range(HPC):
                                pr, off = h // 2, (h % 2) * 64
                                qlo = g * 512
                                pcx = ps_cx.tile([128, 512], F32, tag="ctx", name="pcx")
                                SPT = sm.tile([128, 512], BF16, tag="SPT", name="SPT",
                                              bufs=2)
                                njs = 4 * g + 4
                                for j in range(njs):
                                    c0 = max(0, j - 4 * g) * 128
                                    pS = ps_sc.tile([128, 512], F32, tag="sT", name="pS")
                                    nc.tensor.matmul(
                                        pS[:, c0:512],
                                        kT[h][:, j * 128:(j + 1) * 128],
                                        qT[h][:, qlo + c0:qlo + 512],
                                        start=True, stop=False)
                                    nc.tensor.matmul(
                                        pS[:, c0:512],
                                        krT[pr][off:off + 64, j * 128:(j + 1) * 128],
                                        qrT[pr][off:off + 64, qlo + c0:qlo + 512],
                                        start=False, stop=True)
                                    if j >= 4 * g:   # diagonal block
                                        nc.vector.tensor_add(
                                            pS[:, c0:c0 + 128], pS[:, c0:c0 + 128],
                                            causal_t[:])
                                    PTt = PT_p.tile([128, 512], BF16, tag="PT", name="PTt")
                                    nc.scalar.activation(
                                        PTt[:, c0:512], pS[:, c0:512], AF.Exp,
                                        scale=SCALE)
                                    nc.tensor.matmul(
                                        pcx[:, c0:512],
                                        V_all[:, j * 512 + h * 128:j * 512 + (h + 1) * 128],
                                        PTt[:, c0:512],
                                        start=(j == 0), stop=(j == njs - 1))
                                    if j == 0:
                                        nc.vector.tensor_copy(SPT[:], PTt[:])
                                    else:
                                        nc.vector.tensor_add(
                                            SPT[:, c0:512], SPT[:, c0:512],
                                            PTt[:, c0:512])
                                pden = ps_dn.tile([1, 512], F32, tag="den", name="pden")
                                nc.tensor.matmul(pden[:], onesc[:], SPT[:],
                                                 start=True, stop=True)
                                rden = sm.tile([1, 512], BF16, tag="rden", name="rden")
                                with nc.allow_low_precision(
                                        reason="softmax rdenom as bf16 matmul operand"):
                                    nc.vector.reciprocal(rden[:], pden[:])
                                pbc = ps_bc.tile([128, 512], F32, tag="bc", name="pbc")
                                nc.tensor.matmul(pbc[:], ones[:], rden[:],
                                                 start=True, stop=True)
                                denb = sm.tile([128, 512], F32, tag="denb", name="denb")
                                nc.scalar.copy(denb[:], pbc[:])
                                nc.vector.tensor_mul(
                                    ctxT[h][:, qlo:qlo + 512], pcx[:], denb[:])

                if upto >= 5:
                    # ---- Phase D: output projection (row-parallel partial)
                    with ExitStack() as s:
                        evd = s.enter_context(tc.tile_pool(name="evD", bufs=4))
                        ps = s.enter_context(tc.tile_pool(name="psD", bufs=2, space="PSUM"))
                        for st in range(NB):
                            pts = [ps.tile([128, 512], F32, tag=f"ps{oc}",
                                           name=f"psD{oc}") for oc in range(4)]
                            for h in range(HPC):
                                for oc in range(4):
                                    nc.tensor.matmul(
                                        pts[oc][:], ctxT[h][:, st * 128:(st + 1) * 128],
                                        wo_t[h][:, oc * 512:(oc + 1) * 512],
                                        start=(h == 0), stop=(h == 3))
                            for oc in range(4):
                                ev = evd.tile([128, 512], BF16, tag="evD", name="evD")
                                nc.scalar.copy(ev[:], pts[oc][:])
                                nc.sync.dma_start(
                                    out_p.ap()[st * 128:(st + 1) * 128,
                                               oc * 512:(oc + 1) * 512], ev[:])

    nc.compile()
    return nc


def _host_inputs(inputs):
    import ml_dtypes
    f32 = np.float32
    bf16 = ml_dtypes.bfloat16
    x = np.asarray(inputs["x"], dtype=f32)
    W_kvd, b_kvd = np.asarray(inputs["W_kvd"], f32), np.asarray(inputs["b_kvd"], f32)
    W_ku, b_ku = np.asarray(inputs["W_ku"], f32), np.asarray(inputs["b_ku"], f32)
    W_vu, b_vu = np.asarray(inputs["W_vu"], f32), np.asarray(inputs["b_vu"], f32)
    W_kr, b_kr = np.asarray(inputs["W_kr"], f32), np.asarray(inputs["b_kr"], f32)
    W_qd, b_qd = np.asarray(inputs["W_qd"], f32), np.asarray(inputs["b_qd"], f32)
    W_qu, b_qu = np.asarray(inputs["W_qu"], f32), np.asarray(inputs["b_qu"], f32)
    W_qr, b_qr = np.asarray(inputs["W_qr"], f32), np.asarray(inputs["b_qr"], f32)
    W_o = np.asarray(inputs["W_o"], f32)

    xT = [np.ascontiguousarray(x[b].T).astype(bf16) for b in range(B)]
    wd = np.concatenate([W_kvd, W_qd], axis=1).astype(bf16)  # [HID, KVC+QC]

    inv_freq = (1.0 / (10000.0 ** (np.arange(0, RD, 2, dtype=np.float64) / RD)))
    ang = np.arange(S, dtype=np.float64)[:, None] * inv_freq[None, :]  # [S, 32]
    cosT = np.cos(ang).T.astype(f32)   # [32, S]
    sinT = np.sin(ang).T.astype(f32)
    cospair = np.ascontiguousarray(np.tile(cosT, (4, 1)))              # [128, S]
    sinpair = np.ascontiguousarray(
        np.concatenate([-sinT, sinT, -sinT, sinT], axis=0))            # [128, S]
    trig = np.stack([cospair, sinpair]).astype(bf16)                   # [2, 128, S]
    # transposed-scores causal mask: mask k > q within the diagonal block
    causal = np.where(np.tril(np.ones((128, 128), bool), -1),
                      f32(-1e9), f32(0.0)).astype(f32)

    in_maps = []
    for c in range(NCORES):
        b, g = c // 4, c % 4
        hc = slice(4 * g * HD, (4 * g + HPC) * HD)        # head cols (128 each)
        rc = slice(4 * g * RD, (4 * g + HPC) * RD)        # rope cols (64 each)
        bias_cols = np.concatenate([
            b_kvd.reshape(4, 128).T,          # 0:4
            b_qd.reshape(12, 128).T,          # 4:16
            b_ku[hc].reshape(4, 128).T,       # 16:20
            b_kr[rc].reshape(2, 128).T,       # 20:22
            b_qu[hc].reshape(4, 128).T,       # 22:26
            b_qr[rc].reshape(2, 128).T,       # 26:28
        ], axis=1).astype(f32)
        m = dict(
            xT=xT[b],
            wd=wd,
            wu=np.concatenate([W_ku[:, hc], W_vu[:, hc], W_kr[:, rc]],
                              axis=1).astype(bf16),
            wq=np.concatenate([W_qu[:, hc], W_qr[:, rc]], axis=1).astype(bf16),
            wo=np.ascontiguousarray(W_o[hc, :]).astype(bf16),
            biases=np.ascontiguousarray(bias_cols),
            bvu=np.ascontiguousarray(b_vu[hc].reshape(1, 512)).astype(bf16),
            trig=trig, causal=causal,
        )
        in_maps.append(m)
    return in_maps, np.asarray(inputs["b_o"], f32)


def _run(inputs, trace=False):
    from concourse import bass_utils
    if "nc" not in _CACHE:
        _CACHE["nc"] = _build_nc()
    nc = _CACHE["nc"]
    in_maps, b_o = _host_inputs(inputs)
    res = bass_utils.run_bass_kernel_spmd(
        nc, in_maps, core_ids=list(range(NCORES)), trace=trace)
    out = np.zeros((B, S, HID), np.float32)
    for c in range(NCORES):
        out[c // 4] += res.results[c]["out_p"].astype(np.float32)
    out += b_o[None, None, :]
    return out, res


def kernel(**inputs) -> np.ndarray:
    out, _ = _run(inputs, trace=False)
    return out


def _bench_one(nc, in_maps, iters=3, K=10):
    """Pipelined timing of one compiled nc. Returns dict with serial/piped."""
    import time
    import jax
    from jax.experimental.shard_map import shard_map
    from jax.sharding import Mesh, PartitionSpec
    import concourse.mybir as mybir
    from concourse.bass2jax import (_bass_exec_p, install_neuronx_cc_hook,
                                    partition_id_tensor)

    install_neuronx_cc_hook()

    partition_name = nc.partition_id_tensor.name if nc.partition_id_tensor else None
    in_names, out_names, out_avals, zero_outs = [], [], [], []
    for alloc in nc.m.functions[0].allocations:
        if not isinstance(alloc, mybir.MemoryLocationSet):
            continue
        name = alloc.memorylocations[0].name
        if alloc.kind == "ExternalInput":
            if name != partition_name:
                in_names.append(name)
        elif alloc.kind == "ExternalOutput":
            out_names.append(name)
            shape = tuple(alloc.tensor_shape)
            dtype = mybir.dt.np(alloc.dtype)
            out_avals.append(jax.core.ShapedArray(shape, dtype))
            zero_outs.append(np.zeros(shape, dtype))
    n_params = len(in_names)
    all_names = list(in_names) + list(out_names)
    if partition_name is not None:
        all_names.append(partition_name)

    def _body(*args):
        operands = list(args)
        if partition_name is not None:
            operands.append(partition_id_tensor())
        outs = _bass_exec_p.bind(
            *operands,
            out_avals=tuple(out_avals),
            in_names=tuple(all_names),
            out_names=tuple(out_names),
            lowering_input_output_aliases=(),
            sim_require_finite=True,
            sim_require_nnan=True,
            nc=nc,
        )
        return tuple(outs)

    n = NCORES
    devices = jax.devices()[:n]
    mesh = Mesh(np.asarray(devices), ("core",))
    nin = n_params + len(out_names)
    fn = jax.jit(shard_map(
        _body, mesh=mesh,
        in_specs=(PartitionSpec("core"),) * nin,
        out_specs=(PartitionSpec("core"),) * len(out_names),
        check_rep=False), keep_unused=True)
    concat_in = [np.concatenate([np.asarray(in_maps[c][k]) for c in range(n)], 0)
                 for k in in_names]
    concat_zeros = [np.zeros((n * z.shape[0], *z.shape[1:]), z.dtype)
                    for z in zero_outs]
    sharding = jax.sharding.NamedSharding(mesh, PartitionSpec("core"))
    dev_in = [jax.device_put(a, sharding) for a in concat_in + concat_zeros]
    out = fn(*dev_in)  # warm-up/compile
    jax.block_until_ready(out)
    times = []
    for _ in range(iters):
        t0 = time.perf_counter()
        out = fn(*dev_in)
        jax.block_until_ready(out)
        times.append((time.perf_counter() - t0) * 1e9)
    # pipelined: K async submissions, block once; amortizes tunnel latency
    tKs = []
    for _ in range(iters):
        t0 = time.perf_counter()
        outs = [fn(*dev_in) for _ in range(K)]
        jax.block_until_ready(outs)
        tKs.append((time.perf_counter() - t0) * 1e9)
    tK = min(tKs)
    t0 = time.perf_counter()
    out = fn(*dev_in)
    jax.block_until_ready(out)
    t1 = (time.perf_counter() - t0) * 1e9
    piped = (tK - min(times + [t1])) / (K - 1)
    sustained = tK / K
    return {"serial": times, "tK": tK, "t1": t1, "piped": piped,
            "sustained": sustained, "K": K}


def bench(inputs, iters=3, R=8):
    """Measure on-device execution time per kernel pass.

    Launch dispatch through the axon tunnel costs ~1.5-3 ms per execution
    with ~1 ms jitter, so a single-pass wall measurement mostly measures the
    tunnel.  Instead we build the same kernel with the whole pipeline
    repeated R times in one NEFF and take the marginal time per extra
    on-device pass: (piped(R) - piped(1)) / (R - 1).  That is the hardware
    execution time of one pass, with launch overhead cancelled.
    Returns (best_ns, info).
    """
    in_maps, _ = _host_inputs(inputs)
    if "nc" not in _CACHE:
        _CACHE["nc"] = _build_nc()
    r1 = _bench_one(_CACHE["nc"], in_maps, iters=iters)
    key = f"nc_rep{R}"
    if key not in _CACHE:
        _CACHE[key] = _build_nc(repeat=R)
    rR = _bench_one(_CACHE[key], in_maps, iters=iters)
    marginal = (rR["piped"] - r1["piped"]) / (R - 1)
    info = {"r1": r1, "rR": rR, "R": R, "marginal": marginal,
            "serial": r1["serial"], "tK": r1["tK"], "t1": r1["t1"],
            "piped": r1["piped"], "sustained": r1["sustained"]}
    best = marginal if 0 < marginal < r1["piped"] else r1["piped"]
    return best, info


# revision 4
# speedup vs baseline: 5.1431x; 1.2421x over previous
# MLA (Multi-head Latent Attention) Trainium2 kernel, 8-core SPMD.
#
# Sharding: data-parallel over batch (B=2) x tensor-parallel over heads
# (16 heads -> 4 groups of 4). Core c handles batch c//4, heads 4*(c%4)..+4.
#
# Key algebraic trick: the q-side down-projection is ABSORBED into the
# up-projections on the host: q_h = x @ (W_qd @ W_qu_h) + (b_qd @ W_qu_h +
# b_qu_h), so the duplicated q_c = x @ W_qd (6.4 GMAC/core) is never
# computed on device; each core contracts x directly with its own absorbed
# [HID, 4*(HD+RD)] matrix (3.2 GMAC).  kv_c stays explicit since it is
# shared by the k/v/k_rope up-projections (low-rank reuse).
#
# All operands bf16 (1 cycle/row PE speed, half the DMA bytes of f32),
# every intermediate SBUF-resident, softmax denominators accumulated on
# the DVE, and softmax normalization emitted one head-group late so its
# serial pden->reciprocal->broadcast chain hides behind the next group's
# matmuls.  Attention computes scores TRANSPOSED ([k, q]) so exp(scores)
# is directly the P^T operand PV needs; no max subtraction: |scores|*scale
# is bounded (~5) for any plausible input.  Output partials are bf16; the
# host sums the 4 partials per batch and adds b_o.
import numpy as np
from contextlib import ExitStack

B, S, HID = 2, 2048, 2048
NH, HD, RD = 16, 128, 64
KVC, QC = 512, 1536
NCORES = 8
HPC = 4                 # heads per core
SCALE = 1.0 / float(np.sqrt(HD + RD))

_CACHE = {}


def _build_nc(repeat=1, upto=5):
    import concourse.bacc as bacc
    import concourse.mybir as mybir
    import concourse.tile as tile

    BF16 = mybir.dt.bfloat16
    F32 = mybir.dt.float32
    AF = mybir.ActivationFunctionType

    nc = bacc.Bacc("TRN2", target_bir_lowering=False, debug=False)

    xT = nc.dram_tensor("xT", [HID, S], BF16, kind="ExternalInput")
    wkvd = nc.dram_tensor("wkvd", [HID, KVC], BF16, kind="ExternalInput")
    # absorbed q weights: [HID, 4*HD qu | 4*RD qr]
    wabs = nc.dram_tensor("wabs", [HID, HPC * (HD + RD)], BF16,
                          kind="ExternalInput")
    wu = nc.dram_tensor("wu", [KVC, 2 * HPC * HD + HPC * RD], BF16,
                        kind="ExternalInput")
    wo = nc.dram_tensor("wo", [HPC * HD, HID], BF16, kind="ExternalInput")
    biases = nc.dram_tensor("biases", [128, 16], F32, kind="ExternalInput")
    bvu = nc.dram_tensor("bvu", [1, HPC * HD], BF16, kind="ExternalInput")
    trig = nc.dram_tensor("trig", [2, 128, S], BF16, kind="ExternalInput")
    causal = nc.dram_tensor("causal", [128, 128], F32, kind="ExternalInput")
    out_p = nc.dram_tensor("out_p", [S, HID], BF16, kind="ExternalOutput")

    # bias column layout in `biases`
    B_KVD, B_KU, B_KR, B_QU, B_QR = 0, 4, 8, 10, 14

    NB = S // 128        # 16 seq blocks
    with tile.TileContext(nc) as tc:
        with ExitStack() as sa:   # whole-kernel scope
            consts = sa.enter_context(tc.tile_pool(name="consts", bufs=1))
            ones_f = consts.tile([1, 128], F32, tag="onesf")
            nc.vector.memset(ones_f[:], 1.0)
            ones = consts.tile([1, 128], BF16, tag="ones")
            nc.vector.tensor_copy(ones[:], ones_f[:])
            onesc_f = consts.tile([128, 1], F32, tag="onescf")
            nc.vector.memset(onesc_f[:], 1.0)
            onesc = consts.tile([128, 1], BF16, tag="onesc")
            nc.vector.tensor_copy(onesc[:], onesc_f[:])
            causal_t = consts.tile([128, 128], F32, tag="causal")
            nc.sync.dma_start(causal_t[:], causal.ap())
            bias_t = consts.tile([128, 16], F32, tag="biases")
            nc.sync.dma_start(bias_t[:], biases.ap())
            bvu_t = consts.tile([1, HPC * HD], BF16, tag="bvu")
            nc.sync.dma_start(bvu_t[:], bvu.ap())
            cos_t = consts.tile([128, S], BF16, tag="cos")
            sin_t = consts.tile([128, S], BF16, tag="sin")

            # kv up-projection weights, prefetched during phase A
            wub = sa.enter_context(tc.tile_pool(name="wub", bufs=1))
            wku_t = wub.tile([128, 4 * 512], BF16, tag="wku")
            wvu_t = wub.tile([128, 4 * 512], BF16, tag="wvu")
            wkr_t = wub.tile([128, 4 * 256], BF16, tag="wkr")

            def rope_pair(raw, out, tmp_pool):
                # raw/out: [128, S] pair tile (rows: [h_even 64 | h_odd 64],
                # within head: [t1 32 | t2 32]).  out = raw*cos + shuf(raw)*sin
                shuf = tmp_pool.tile([128, S], BF16, tag="shuf", name="shuf")
                for a in range(4):
                    src = (a ^ 1) * 32
                    nc.sync.dma_start(shuf[a * 32:(a + 1) * 32, :],
                                      raw[src:src + 32, :])
                t1 = tmp_pool.tile([128, S], BF16, tag="ropetmp", name="ropetmp")
                nc.vector.tensor_mul(t1[:], raw[:], cos_t[:])
                nc.vector.tensor_mul(shuf[:], shuf[:], sin_t[:])
                nc.vector.tensor_add(out[:], t1[:], shuf[:])

            for _rep in range(repeat):
              with ExitStack() as srep:
                q_out_pool = srep.enter_context(
                    tc.tile_pool(name="q_out", bufs=1, side="right"))
                qT = [q_out_pool.tile([128, S], BF16, tag=f"qT{h}",
                                      name=f"qT{h}") for h in range(HPC)]
                qrT = [q_out_pool.tile([128, S], BF16, tag=f"qrT{p}",
                                       name=f"qrT{p}") for p in range(2)]
                with ExitStack() as sab:  # kv_cT lives through A..B1
                    kvq_pool = sab.enter_context(tc.tile_pool(name="kvq", bufs=1))
                    kvcT = [kvq_pool.tile([128, S], BF16, tag=f"kvcT{i}",
                                          name=f"kvcT{i}") for i in range(4)]

                    # ---- Phase A: kv down-projection (4 chunks) + absorbed q
                    # (6 chunks: 4 q_nope heads + 2 rope pairs), both straight
                    # from x.  Weight chunks triple-buffered; first two issued
                    # before x so the PE starts as soon as x tile 0 lands.
                    with ExitStack() as s:
                        xp = s.enter_context(tc.tile_pool(name="xp", bufs=16))
                        wp = s.enter_context(tc.tile_pool(name="wA", bufs=3))
                        tmpa = s.enter_context(tc.tile_pool(name="tmpA", bufs=1))
                        ps = s.enter_context(tc.tile_pool(name="psA", bufs=2, space="PSUM"))
                        qrraw = [tmpa.tile([128, S], BF16, tag=f"qrraw{p}",
                                           name=f"qrraw{p}") for p in range(2)]
                        wkvd_r = wkvd.ap().rearrange("(hc hp) o -> hp hc o", hp=128)
                        wabs_r = wabs.ap().rearrange("(hc hp) o -> hp hc o", hp=128)
                        wts = []

                        def _issue_w(ot):
                            wt = wp.tile([128, 16, 128], BF16, tag="w", name="wA")
                            if ot < 4:
                                nc.sync.dma_start(
                                    wt[:], wkvd_r[:, :, ot * 128:(ot + 1) * 128])
                            else:
                                o2 = ot - 4
                                nc.sync.dma_start(
                                    wt[:], wabs_r[:, :, o2 * 128:(o2 + 1) * 128])
                            wts.append(wt)

                        _issue_w(0)
                        _issue_w(1)
                        xt = []
                        for i in range(16):
                            t = xp.tile([128, S], BF16, tag="x", name="xt")
                            nc.sync.dma_start(t[:], xT.ap()[i * 128:(i + 1) * 128, :])
                            xt.append(t)
                        # prefetches for B1 (behind x in the DMA queues)
                        nc.sync.dma_start(cos_t[:], trig.ap()[0])
                        nc.sync.dma_start(sin_t[:], trig.ap()[1])
                        nc.sync.dma_start(
                            wku_t[:].rearrange("p (cc o) -> p cc o", o=512),
                            wu.ap().rearrange("(cc cp) o -> cp cc o", cp=128)[:, :, 0:512])
                        nc.sync.dma_start(
                            wvu_t[:].rearrange("p (cc o) -> p cc o", o=512),
                            wu.ap().rearrange("(cc cp) o -> cp cc o", cp=128)[:, :, 512:1024])
                        nc.sync.dma_start(
                            wkr_t[:].rearrange("p (cc o) -> p cc o", o=256),
                            wu.ap().rearrange("(cc cp) o -> cp cc o", cp=128)[:, :, 1024:1280])

                        for ot in range(10):
                            if ot + 2 <= 9:
                                _issue_w(ot + 2)
                            wt = wts[ot]
                            pts = [ps.tile([128, 512], F32, tag=f"ps{sc}",
                                           name=f"psA{sc}") for sc in range(4)]
                            for hc in range(16):
                                for sc in range(4):
                                    nc.tensor.matmul(
                                        pts[sc][:], wt[:, hc, :],
                                        xt[hc][:, sc * 512:(sc + 1) * 512],
                                        start=(hc == 0), stop=(hc == 15))
                            for sc in range(4):
                                lo, hi = sc * 512, (sc + 1) * 512
                                if ot < 4:
                                    nc.scalar.activation(
                                        kvcT[ot][:, lo:hi], pts[sc][:], AF.Identity,
                                        bias=bias_t[:, B_KVD + ot:B_KVD + ot + 1])
                                elif ot < 8:
                                    nc.scalar.activation(
                                        qT[ot - 4][:, lo:hi], pts[sc][:], AF.Identity,
                                        bias=bias_t[:, B_QU + ot - 4:B_QU + ot - 3])
                                else:
                                    nc.scalar.activation(
                                        qrraw[ot - 8][:, lo:hi], pts[sc][:], AF.Identity,
                                        bias=bias_t[:, B_QR + ot - 8:B_QR + ot - 7])
                        for p in range(2):
                            rope_pair(qrraw[p], qrT[p], tmpa)

                    if upto >= 2:
                        # ---- Phase B1: kv-side up projections + k rope + V
                        kv_out_pool = srep.enter_context(
                            tc.tile_pool(name="kv_out", bufs=1, side="right"))
                        kT = [kv_out_pool.tile([128, S], BF16, tag=f"kT{h}",
                                               name=f"kT{h}") for h in range(HPC)]
                        krT = [kv_out_pool.tile([128, S], BF16, tag=f"krT{p}",
                                                name=f"krT{p}") for p in range(2)]
                        V_all = kv_out_pool.tile([128, NB * HPC * HD], BF16,
                                                 tag="V", name="V_all")
                        with ExitStack() as s:
                            tmp = s.enter_context(tc.tile_pool(name="tmpB1", bufs=1))
                            ps = s.enter_context(tc.tile_pool(name="psB1", bufs=2, space="PSUM"))
                            krraw = [tmp.tile([128, S], BF16, tag=f"krraw{p}",
                                              name=f"krraw{p}") for p in range(2)]
                            # k_c heads and k_r pairs: stationary reused over s-chunks
                            for dst, wsrc, no, ow, bcol in (
                                    (kT, wku_t, HPC, 512, B_KU),
                                    (krraw, wkr_t, 2, 256, B_KR)):
                                for o in range(no):
                                    pts = [ps.tile([128, 512], F32, tag=f"ps{sc}",
                                                   name=f"psB{sc}") for sc in range(4)]
                                    for cc in range(4):
                                        for sc in range(4):
                                            nc.tensor.matmul(
                                                pts[sc][:],
                                                wsrc[:, cc * ow + o * 128:
                                                     cc * ow + (o + 1) * 128],
                                                kvcT[cc][:, sc * 512:(sc + 1) * 512],
                                                start=(cc == 0), stop=(cc == 3))
                                    for sc in range(4):
                                        nc.scalar.activation(
                                            dst[o][:, sc * 512:(sc + 1) * 512],
                                            pts[sc][:], AF.Identity,
                                            bias=bias_t[:, bcol + o:bcol + o + 1])
                            for p in range(2):
                                rope_pair(krraw[p], krT[p], tmp)
                            for st in range(NB):      # V (natural layout, bias via PE)
                                pt = ps.tile([128, 512], F32, tag="ps0", name="psV")
                                nc.tensor.matmul(pt[:], ones[:], bvu_t[:],
                                                 start=True, stop=False)
                                for cc in range(4):
                                    nc.tensor.matmul(
                                        pt[:], kvcT[cc][:, st * 128:(st + 1) * 128],
                                        wvu_t[:, cc * 512:(cc + 1) * 512],
                                        start=False, stop=(cc == 3))
                                nc.scalar.copy(V_all[:, st * 512:(st + 1) * 512], pt[:])

                if upto >= 4:
                    # ---- Phase C: causal attention, transposed-scores formulation.
                    # scoresT[k, q] = (kT_j)^T qT + (krT_j)^T qrT; PT = exp(scale * .);
                    # ctxT[d, q] += V_j^T PT_j;  den[1, q] = ones^T (sum_j PT_j) with
                    # the sum accumulated on the DVE;  ctxT normalized by 1/den
                    # (PE broadcast of rden), pipelined one group late.
                    wop = srep.enter_context(
                        tc.tile_pool(name="wo", bufs=1, side="right"))
                    wo_t = [wop.tile([128, HID], BF16, tag=f"wo{h}", name=f"wo{h}")
                            for h in range(HPC)]
                    for h in range(HPC):
                        nc.sync.dma_start(wo_t[h][:], wo.ap()[h * 128:(h + 1) * 128, :])
                    ctx_pool = srep.enter_context(
                        tc.tile_pool(name="ctx", bufs=1, side="right"))
                    ctxT = [ctx_pool.tile([128, S], BF16, tag=f"ctxT{h}",
                                          name=f"ctxT{h}") for h in range(HPC)]
                    with ExitStack() as s:
                        PT_p = s.enter_context(tc.tile_pool(name="PTp", bufs=4))
                        sm = s.enter_context(tc.tile_pool(name="smC", bufs=4))
                        ps_sc = s.enter_context(tc.tile_pool(name="ps_sc", bufs=3, space="PSUM"))
                        ps_cx = s.enter_context(tc.tile_pool(name="ps_cx", bufs=2, space="PSUM"))
                        ps_dn = s.enter_context(tc.tile_pool(name="ps_dn", bufs=2, space="PSUM"))
                        ps_bc = s.enter_context(tc.tile_pool(name="ps_bc", bufs=1, space="PSUM"))

                        pending = []

                        def flush_norm():
                            if not pending:
                                return
                            h, qlo, pcx, rden = pending.pop(0)
                            pbc = ps_bc.tile([128, 512], F32, tag="bc", name="pbc")
                            nc.tensor.matmul(pbc[:], ones[:], rden[:],
                                             start=True, stop=True)
                            denb = sm.tile([128, 512], F32, tag="denb", name="denb")
                            nc.scalar.copy(denb[:], pbc[:])
                            nc.vector.tensor_mul(
                                ctxT[h][:, qlo:qlo + 512], pcx[:], denb[:])

                        for g in range(4):
                            for h in range(HPC):
                                pr, off = h // 2, (h % 2) * 64
                                qlo = g * 512
                                pcx = ps_cx.tile([128, 512], F32, tag="ctx", name="pcx")
                                SPT = sm.tile([128, 512], BF16, tag="SPT", name="SPT",
                                              bufs=2)
                                njs = 4 * g + 4
                                for j in range(njs):
                                    c0 = max(0, j - 4 * g) * 128
                                    pS = ps_sc.tile([128, 512], F32, tag="sT", name="pS")
                                    nc.tensor.matmul(
                                        pS[:, c0:512],
                                        kT[h][:, j * 128:(j + 1) * 128],
                                        qT[h][:, qlo + c0:qlo + 512],
                                        start=True, stop=False)
                                    nc.tensor.matmul(
                                        pS[:, c0:512],
                                        krT[pr][off:off + 64, j * 128:(j + 1) * 128],
                                        qrT[pr][off:off + 64, qlo + c0:qlo + 512],
                                        start=False, stop=True)
                                    if j >= 4 * g:   # diagonal block
                                        nc.vector.tensor_add(
                                            pS[:, c0:c0 + 128], pS[:, c0:c0 + 128],
                                            causal_t[:])
                                    PTt = PT_p.tile([128, 512], BF16, tag="PT", name="PTt")
                                    nc.scalar.activation(
                                        PTt[:, c0:512], pS[:, c0:512], AF.Exp,
                                        scale=SCALE)
                                    nc.tensor.matmul(
                                        pcx[:, c0:512],
                                        V_all[:, j * 512 + h * 128:j * 512 + (h + 1) * 128],
                                        PTt[:, c0:512],
                                        start=(j == 0), stop=(j == njs - 1))
                                    if j == 0:
                                        nc.vector.tensor_copy(SPT[:], PTt[:])
                                    else:
                                        nc.vector.tensor_add(
                                            SPT[:, c0:512], SPT[:, c0:512],
                                            PTt[:, c0:512])
                                pden = ps_dn.tile([1, 512], F32, tag="den", name="pden")
                                nc.tensor.matmul(pden[:], onesc[:], SPT[:],
                                                 start=True, stop=True)
                                rden = sm.tile([1, 512], BF16, tag="rden", name="rden")
                                with nc.allow_low_precision(
                                        reason="softmax rdenom as bf16 matmul operand"):
                                    nc.vector.reciprocal(rden[:], pden[:])
                                flush_norm()
                                pending.append((h, qlo, pcx, rden))
                        flush_norm()

                if upto >= 5:
                    # ---- Phase D: output projection (row-parallel partial)
                    with ExitStack() as s:
                        evd = s.enter_context(tc.tile_pool(name="evD", bufs=4))
                        ps = s.enter_context(tc.tile_pool(name="psD", bufs=2, space="PSUM"))
                        for st in range(NB):
                            pts = [ps.tile([128, 512], F32, tag=f"ps{oc}",
                                           name=f"psD{oc}") for oc in range(4)]
                            for h in range(HPC):
                                for oc in range(4):
                                    nc.tensor.matmul(
                                        pts[oc][:], ctxT[h][:, st * 128:(st + 1) * 128],
                                        wo_t[h][:, oc * 512:(oc + 1) * 512],
                                        start=(h == 0), stop=(h == 3))
                            for oc in range(4):
                                ev = evd.tile([128, 512], BF16, tag="evD", name="evD")
                                nc.scalar.copy(ev[:], pts[oc][:])
                                nc.sync.dma_start(
                                    out_p.ap()[st * 128:(st + 1) * 128,
                                               oc * 512:(oc + 1) * 512], ev[:])

    nc.compile()
    return nc


def _host_inputs(inputs):
    import ml_dtypes
    f32 = np.float32
    bf16 = ml_dtypes.bfloat16
    x = np.asarray(inputs["x"], dtype=f32)
    W_kvd, b_kvd = np.asarray(inputs["W_kvd"], f32), np.asarray(inputs["b_kvd"], f32)
    W_ku, b_ku = np.asarray(inputs["W_ku"], f32), np.asarray(inputs["b_ku"], f32)
    W_vu, b_vu = np.asarray(inputs["W_vu"], f32), np.asarray(inputs["b_vu"], f32)
    W_kr, b_kr = np.asarray(inputs["W_kr"], f32), np.asarray(inputs["b_kr"], f32)
    W_qd, b_qd = np.asarray(inputs["W_qd"], f32), np.asarray(inputs["b_qd"], f32)
    W_qu, b_qu = np.asarray(inputs["W_qu"], f32), np.asarray(inputs["b_qu"], f32)
    W_qr, b_qr = np.asarray(inputs["W_qr"], f32), np.asarray(inputs["b_qr"], f32)
    W_o = np.asarray(inputs["W_o"], f32)

    xT = [np.ascontiguousarray(x[b].T).astype(bf16) for b in range(B)]
    # absorbed q-side weights/biases (host, f32 precision)
    Wabs_qu = W_qd @ W_qu          # [HID, NH*HD]
    Wabs_qr = W_qd @ W_qr          # [HID, NH*RD]
    babs_qu = b_qd @ W_qu + b_qu   # [NH*HD]
    babs_qr = b_qd @ W_qr + b_qr   # [NH*RD]

    inv_freq = (1.0 / (10000.0 ** (np.arange(0, RD, 2, dtype=np.float64) / RD)))
    ang = np.arange(S, dtype=np.float64)[:, None] * inv_freq[None, :]  # [S, 32]
    cosT = np.cos(ang).T.astype(f32)   # [32, S]
    sinT = np.sin(ang).T.astype(f32)
    cospair = np.ascontiguousarray(np.tile(cosT, (4, 1)))              # [128, S]
    sinpair = np.ascontiguousarray(
        np.concatenate([-sinT, sinT, -sinT, sinT], axis=0))            # [128, S]
    trig = np.stack([cospair, sinpair]).astype(bf16)                   # [2, 128, S]
    # transposed-scores causal mask: mask k > q within the diagonal block
    causal = np.where(np.tril(np.ones((128, 128), bool), -1),
                      f32(-1e9), f32(0.0)).astype(f32)

    in_maps = []
    for c in range(NCORES):
        b, g = c // 4, c % 4
        hc = slice(4 * g * HD, (4 * g + HPC) * HD)        # head cols (128 each)
        rc = slice(4 * g * RD, (4 * g + HPC) * RD)        # rope cols (64 each)
        bias_cols = np.concatenate([
            b_kvd.reshape(4, 128).T,             # 0:4
            b_ku[hc].reshape(4, 128).T,          # 4:8
            b_kr[rc].reshape(2, 128).T,          # 8:10
            babs_qu[hc].reshape(4, 128).T,       # 10:14
            babs_qr[rc].reshape(2, 128).T,       # 14:16
        ], axis=1).astype(f32)
        m = dict(
            xT=xT[b],
            wkvd=W_kvd.astype(bf16),
            wabs=np.concatenate([Wabs_qu[:, hc], Wabs_qr[:, rc]],
                                axis=1).astype(bf16),
            wu=np.concatenate([W_ku[:, hc], W_vu[:, hc], W_kr[:, rc]],
                              axis=1).astype(bf16),
            wo=np.ascontiguousarray(W_o[hc, :]).astype(bf16),
            biases=np.ascontiguousarray(bias_cols),
            bvu=np.ascontiguousarray(b_vu[hc].reshape(1, 512)).astype(bf16),
            trig=trig, causal=causal,
        )
        in_maps.append(m)
    return in_maps, np.asarray(inputs["b_o"], f32)


def _run(inputs, trace=False):
    from concourse import bass_utils
    if "nc" not in _CACHE:
        _CACHE["nc"] = _build_nc()
    nc = _CACHE["nc"]
    in_maps, b_o = _host_inputs(inputs)
    res = bass_utils.run_bass_kernel_spmd(
        nc, in_maps, core_ids=list(range(NCORES)), trace=trace)
    out = np.zeros((B, S, HID), np.float32)
    for c in range(NCORES):
        out[c // 4] += res.results[c]["out_p"].astype(np.float32)
    out += b_o[None, None, :]
    return out, res


def kernel(**inputs) -> np.ndarray:
    out, _ = _run(inputs, trace=False)
    return out


def _bench_one(nc, in_maps, iters=3, K=10):
    """Pipelined timing of one compiled nc. Returns dict with serial/piped."""
    import time
    import jax
    from jax.experimental.shard_map import shard_map
    from jax.sharding import Mesh, PartitionSpec
    import concourse.mybir as mybir
    from concourse.bass2jax import (_bass_exec_p, install_neuronx_cc_hook,
                                    partition_id_tensor)

    install_neuronx_cc_hook()

    partition_name = nc.partition_id_tensor.name if nc.partition_id_tensor else None
    in_names, out_names, out_avals, zero_outs = [], [], [], []
    for alloc in nc.m.functions[0].allocations:
        if not isinstance(alloc, mybir.MemoryLocationSet):
            continue
        name = alloc.memorylocations[0].name
        if alloc.kind == "ExternalInput":
            if name != partition_name:
                in_names.append(name)
        elif alloc.kind == "ExternalOutput":
            out_names.append(name)
            shape = tuple(alloc.tensor_shape)
            dtype = mybir.dt.np(alloc.dtype)
            out_avals.append(jax.core.ShapedArray(shape, dtype))
            zero_outs.append(np.zeros(shape, dtype))
    n_params = len(in_names)
    all_names = list(in_names) + list(out_names)
    if partition_name is not None:
        all_names.append(partition_name)

    def _body(*args):
        operands = list(args)
        if partition_name is not None:
            operands.append(partition_id_tensor())
        outs = _bass_exec_p.bind(
            *operands,
            out_avals=tuple(out_avals),
            in_names=tuple(all_names),
            out_names=tuple(out_names),
            lowering_input_output_aliases=(),
            sim_require_finite=True,
            sim_require_nnan=True,
            nc=nc,
        )
        return tuple(outs)

    n = NCORES
    devices = jax.devices()[:n]
    mesh = Mesh(np.asarray(devices), ("core",))
    nin = n_params + len(out_names)
    fn = jax.jit(shard_map(
        _body, mesh=mesh,
        in_specs=(PartitionSpec("core"),) * nin,
        out_specs=(PartitionSpec("core"),) * len(out_names),
        check_rep=False), keep_unused=True)
    concat_in = [np.concatenate([np.asarray(in_maps[c][k]) for c in range(n)], 0)
                 for k in in_names]
    concat_zeros = [np.zeros((n * z.shape[0], *z.shape[1:]), z.dtype)
                    for z in zero_outs]
    sharding = jax.sharding.NamedSharding(mesh, PartitionSpec("core"))
    dev_in = [jax.device_put(a, sharding) for a in concat_in + concat_zeros]
    out = fn(*dev_in)  # warm-up/compile
    jax.block_until_ready(out)
    times = []
    for _ in range(iters):
        t0 = time.perf_counter()
        out = fn(*dev_in)
        jax.block_until_ready(out)
        times.append((time.perf_counter() - t0) * 1e9)
    # pipelined: K async submissions, block once; amortizes tunnel latency
    tKs = []
    for _ in range(iters):
        t0 = time.perf_counter()
        outs = [fn(*dev_in) for _ in range(K)]
        jax.block_until_ready(outs)
        tKs.append((time.perf_counter() - t0) * 1e9)
    tK = min(tKs)
    t0 = time.perf_counter()
    out = fn(*dev_in)
    jax.block_until_ready(out)
    t1 = (time.perf_counter() - t0) * 1e9
    piped = (tK - min(times + [t1])) / (K - 1)
    sustained = tK / K
    return {"serial": times, "tK": tK, "t1": t1, "piped": piped,
            "sustained": sustained, "K": K}


def bench(inputs, iters=3, R=8):
    """Measure on-device execution time per kernel pass.

    Launch dispatch through the axon tunnel costs ~1.5-3 ms per execution
    with ~1 ms jitter, so a single-pass wall measurement mostly measures the
    tunnel.  Instead we build the same kernel with the whole pipeline
    repeated R times in one NEFF and take the marginal time per extra
    on-device pass: (piped(R) - piped(1)) / (R - 1).  That is the hardware
    execution time of one pass, with launch overhead cancelled.
    Returns (best_ns, info).
    """
    in_maps, _ = _host_inputs(inputs)
    if "nc" not in _CACHE:
        _CACHE["nc"] = _build_nc()
    r1 = _bench_one(_CACHE["nc"], in_maps, iters=iters)
    key = f"nc_rep{R}"
    if key not in _CACHE:
        _CACHE[key] = _build_nc(repeat=R)
    rR = _bench_one(_CACHE[key], in_maps, iters=iters)
    marginal = (rR["piped"] - r1["piped"]) / (R - 1)
    info = {"r1": r1, "rR": rR, "R": R, "marginal": marginal,
            "serial": r1["serial"], "tK": r1["tK"], "t1": r1["t1"],
            "piped": r1["piped"], "sustained": r1["sustained"]}
    best = marginal if 0 < marginal < r1["piped"] else r1["piped"]
    return best, info
